# revision 1
# baseline (speedup 1.0000x reference)
"""Tensor-parallel GQA multi-head attention (RoPE + causal softmax) for 8 trn2 cores.

Sharding v2: every core handles BOTH batches with 4 q-heads / 1 kv-head:
core c owns q-heads {4c..4c+3} (kv-head c) of batches 0 and 1. Attention
runs in transposed (feature-major) layout with flash-style causal tiling.
Per 512-token slab, the 8 cores exchange their normalized attention outputs
with one AllToAll (bf16, 512KB) so that core c ends up with ALL 2048
attention features for its 128-position output stripe (batch c//4, stripe
c%4); it then applies the full wo to produce disjoint output rows. No
reduction collective is needed.
"""

import sys

sys.path.insert(0, "/opt/trn_rl_repo")

import numpy as np

import concourse.bass as bass
import concourse.bacc as bacc
import concourse.mybir as mybir
from concourse import tile
from concourse.bass_utils import run_bass_kernel_spmd

B, S, D = 2, 2048, 2048
N_HEADS, N_KV, HD = 32, 8, 64
NCORES = 8
QH = 4    # q-heads per core
FQ = QH * HD       # 256 q-feature cols per core
FKV = 2 * HD       # 128 (K then V) per core
SCALE = 1.0 / 8.0  # 1/sqrt(HD)

QTILE = 512
KTILE = 128
NSLAB = S // QTILE  # 4
ND = D // 128       # 16 contraction chunks

F32 = mybir.dt.float32
EXP = mybir.ActivationFunctionType.Exp
BF16 = mybir.dt.bfloat16
MMD = BF16
LE = mybir.AluOpType.is_ge


def _build_kernel(tc, io):
    nc = tc.nc
    xT, wq, wkv, wo = io["xT"], io["wq"], io["wkv"], io["wo"]
    cos2, sin2s, sel = io["cos2"], io["sin2s"], io["sel"]
    out_full = io["out"]
    single = bool(io.get("single"))

    # ---------------- pools ----------------
    const = tc.alloc_tile_pool(name="const", bufs=1)
    wpool = tc.alloc_tile_pool(name="wpool", bufs=1, side="right")
    kvp = tc.alloc_tile_pool(name="kvp", bufs=1)
    xpool = tc.alloc_tile_pool(name="xpool", bufs=2)
    qpool = tc.alloc_tile_pool(name="qpool", bufs=2)
    aop = tc.alloc_tile_pool(name="aop", bufs=2, side="right")
    rp = tc.alloc_tile_pool(name="rp", bufs=2)
    pexp = tc.alloc_tile_pool(name="pexp", bufs=5)
    evac = tc.alloc_tile_pool(name="evac", bufs=2)
    aogp = tc.alloc_tile_pool(name="aogp", bufs=2, side="right")
    dram = tc.alloc_tile_pool(name="dram", bufs=1, space="DRAM")

    psM = tc.alloc_tile_pool(name="psM", bufs=2, space="PSUM")
    psS = tc.alloc_tile_pool(name="psS", bufs=2, space="PSUM")
    psO = tc.alloc_tile_pool(name="psO", bufs=1, space="PSUM")

    # ------- constants + weights; DMA order tuned for fast start -------
    cos2_t = const.tile([128, S], MMD)
    nc.sync.dma_start(cos2_t[:], cos2[:])
    sin2s_t = const.tile([128, S], MMD)
    nc.sync.dma_start(sin2s_t[:], sin2s[:])
    ident = const.tile([128, 64], F32)
    nc.gpsimd.memset(ident[:], 0.0)
    for p in (0, 64):
        nc.gpsimd.affine_select(
            out=ident[p:p + 64, :], in_=ident[p:p + 64, :],
            compare_op=mybir.AluOpType.not_equal,
            fill=1.0, base=0, pattern=[[-1, 64]], channel_multiplier=1,
        )
    # x slab for (b=0, j=0) interleaved with the projection weights, spread
    # over both HW DMA queues so the first matmul group starts within ~5us
    xts00 = []
    WQ = {}
    WKV = {}
    for k in range(ND):
        q1, q2 = (nc.sync, nc.scalar) if k % 2 == 0 else (nc.scalar, nc.sync)
        xt = xpool.tile([128, QTILE], MMD, name="xt", tag=f"xt{k}")
        q1.dma_start(xt[:], xT[k * 128:(k + 1) * 128, 0:QTILE])
        xts00.append(xt)
        for t in range(2):
            w = wpool.tile([128, 128], MMD, name=f"wq{t}_{k}")
            q2.dma_start(w[:], wq[k * 128:(k + 1) * 128,
                                  t * 128:(t + 1) * 128])
            WQ[t, k] = w
        w = wpool.tile([128, 128], MMD, name=f"wkv{k}")
        q1.dma_start(w[:], wkv[k * 128:(k + 1) * 128, :])
        WKV[k] = w

    sel_t = const.tile([2 * QH, 4 * KTILE], MMD)
    nc.sync.dma_start(sel_t[:], sel[:])

    # full wo (loaded via the scalar DMA queue; scalar is idle early on)
    WO = {}
    for fc in range(ND):
        for dn in range(4):
            w = wpool.tile([128, QTILE], MMD, name=f"wo{fc}_{dn}")
            nc.scalar.dma_start(
                w[:], wo[fc * 128:(fc + 1) * 128,
                         dn * QTILE:(dn + 1) * QTILE])
            WO[fc, dn] = w

    # persistent K/V cache tiles
    KK = [kvp.tile([128, S], MMD, name=f"kk{b}") for b in range(B)]
    VA = {}
    for b in range(B):
        for i in range(S // KTILE):
            VA[b, i] = kvp.tile([128, HD + 1], MMD, name=f"va{b}_{i}")

    # A2A dram tiles (one pair per slab)
    a2a_in = [dram.tile([FQ * NCORES, KTILE], MMD, name=f"ain{j}")
              for j in range(NSLAB)]
    a2a_out = [dram.tile([FQ * NCORES, KTILE], MMD, name=f"aout{j}")
               for j in range(NSLAB)]

    AO = {}   # per (b, t) slab-local attention output, feature-major
    QT = {}

    def rope(dst, rows, qs, tab_qs):
        # dst[rows, qs] = dst*cos + swap32(dst)*sin  (feature-major RoPE);
        # qs indexes dst columns, tab_qs the (global-position) rope tables
        n = rows[1] - rows[0]
        qsw = rp.tile([128, QTILE], MMD, name="qsw", tag="qsw")
        for p in range(rows[0], rows[1], 64):
            q0 = p - rows[0]
            nc.sync.dma_start(qsw[q0:q0 + 32, :], dst[p + 32:p + 64, qs])
            nc.sync.dma_start(qsw[q0 + 32:q0 + 64, :], dst[p:p + 32, qs])
        t1 = rp.tile([128, QTILE], F32, name="t1", tag="t1")
        nc.vector.tensor_mul(t1[:n], dst[rows[0]:rows[1], qs],
                             cos2_t[rows[0]:rows[1], tab_qs])
        t2 = rp.tile([128, QTILE], F32, name="t2", tag="t2")
        nc.vector.tensor_mul(t2[:n], qsw[:n], sin2s_t[rows[0]:rows[1], tab_qs])
        nc.vector.tensor_add(dst[rows[0]:rows[1], qs], t1[:n], t2[:n])

    def prefetch_x(b, j):
        qs = slice(j * QTILE, (j + 1) * QTILE)
        xts = []
        for k in range(ND):
            xt = xpool.tile([128, QTILE], MMD, name="xt", tag=f"xt{k}")
            nc.sync.dma_start(
                xt[:], xT[b * D + k * 128:b * D + (k + 1) * 128, qs])
            xts.append(xt)
        return xts

    def make_proj_fillers(b, j, xts):
        # projection for (b, j), split into small PE chunks so it can be
        # woven into the preceding attention's exp-bound inner loop
        qs = slice(j * QTILE, (j + 1) * QTILE)
        ctx = {}
        fillers = []
        if xts is not None:
            ctx["x"] = xts
        else:
            def loadx():
                ctx["x"] = prefetch_x(b, j)
            fillers.append(loadx)
        for f in range(3):
            for sub in range(8):
                def mmchunk(f=f, sub=sub):
                    if sub == 0:
                        ctx[f] = psM.tile([128, QTILE], F32, name="psq",
                                          tag="mm")
                    ps = ctx[f]
                    for k in range(2 * sub, 2 * sub + 2):
                        w = WQ[f, k] if f < 2 else WKV[k]
                        nc.tensor.matmul(ps[:], w[:], ctx["x"][k][:],
                                         start=(k == 0), stop=(k == ND - 1))
                fillers.append(mmchunk)

            def evacf(f=f):
                ps = ctx[f]
                if f < 2:
                    qt = qpool.tile([128, QTILE], MMD, name="qt",
                                    tag=f"qt{b}_{f}")
                    QT[b, f] = qt
                    nc.vector.tensor_copy(qt[:], ps[:])
                    rope(qt, (0, 128), slice(0, QTILE), qs)
                else:
                    nc.vector.tensor_copy(KK[b][0:64, qs], ps[0:64, :])
                    rope(KK[b], (0, 64), qs, qs)
                    # duplicate roped K into rows 64:128 (row-tiled scores)
                    nc.sync.dma_start(KK[b][64:128, qs], KK[b][0:64, qs])
                    vv = rp.tile([128, QTILE], F32, name="vv", tag="vv")
                    nc.vector.tensor_copy(vv[64:128, :], ps[64:128, :])
                    ctx["vv"] = vv
            fillers.append(evacf)
        for c in range(4):
            def vtrans(c=c):
                i = 4 * j + c
                tp = psM.tile([128, QTILE], F32, name="tp", tag="mm")
                vv = ctx["vv"]
                nc.tensor.matmul(tp[:, 0:HD],
                                 vv[64:128, c * 128:(c + 1) * 128],
                                 ident[64:128, :], is_transpose=True,
                                 start=True, stop=True)
                va = VA[b, i]
                nc.vector.tensor_copy(va[:, 0:HD], tp[:, 0:HD])
                nc.vector.memset(va[:, HD:HD + 1], 1.0)
            fillers.append(vtrans)
        return fillers

    def proj(b, j, xts=None):
        for f in make_proj_fillers(b, j, xts):
            f()

    def attn(b, j, fillers=None):
        # fillers: closures emitting small independent PE chunks (wo/proj
        # work); paced evenly through the loop and placed before each attnV
        # so the tensor engine has work while it would otherwise stall on
        # the exp
        fillers = list(fillers or [])
        nkt = 4 * j + 4
        slots = 2 * nkt
        rate = len(fillers) / slots if slots else 0.0
        acc = [0.0]

        def pop_fillers():
            acc[0] += rate
            while fillers and acc[0] >= 1.0:
                fillers.pop(0)()
                acc[0] -= 1.0
        for t in range(2):
            oA = psO.tile([HD + 1, QTILE], F32, name="oA", tag="oA")
            oB = psO.tile([HD + 1, QTILE], F32, name="oB", tag="oB")
            sabs = {}

            def scores(i):
                r = i - 4 * j
                off = max(r, 0) * KTILE
                ks = slice(i * KTILE, (i + 1) * KTILE)
                sAB = psS.tile([128, 2 * QTILE], F32, name="sAB", tag="sAB")
                nc.tensor.matmul(sAB[:, off:QTILE], KK[b][0:64, ks],
                                 QT[b, t][0:64, off:], start=True, stop=True,
                                 tile_position=(0, 0))
                nc.tensor.matmul(sAB[:, QTILE + off:], KK[b][64:128, ks],
                                 QT[b, t][64:128, off:], start=True, stop=True,
                                 tile_position=(64, 0))
                sabs[i] = sAB

            scores(0)
            for i in range(nkt):
                r = i - 4 * j
                off = max(r, 0) * KTILE
                if i + 1 < nkt:
                    scores(i + 1)
                sAB = sabs.pop(i)
                pAB = pexp.tile([128, 2 * QTILE], MMD, name="pAB", tag="pAB")
                nc.scalar.activation(pAB[:, off:], sAB[:, off:], EXP,
                                     scale=SCALE)
                if r >= 0:
                    for h in (off, QTILE + off):
                        # zero the strictly-upper triangle (causal mask);
                        # gpsimd: keeps the DVE queue off this critical path
                        nc.gpsimd.affine_select(
                            out=pAB[:, h:h + KTILE], in_=pAB[:, h:h + KTILE],
                            compare_op=LE, fill=0.0, base=0,
                            pattern=[[1, KTILE]], channel_multiplier=-1)
                pop_fillers()
                nc.tensor.matmul(oA[:, off:], VA[b, i][:], pAB[:, off:QTILE],
                                 start=(i == 0), stop=(i == nkt - 1))
                nc.tensor.matmul(oB[:, off:], VA[b, i][:], pAB[:, QTILE + off:],
                                 start=(i == 0), stop=(i == nkt - 1))
            tA = evac.tile([HD + 1, QTILE], MMD, name="tA", tag="tA",
                           bufs=3)
            tB = evac.tile([HD + 1, QTILE], MMD, name="tB", tag="tB",
                           bufs=3)
            nc.vector.tensor_copy(tA[:], oA[:])
            nc.vector.tensor_copy(tB[:], oB[:])
            ao = AO[b, t]
            nc.sync.dma_start(ao[0:64, :], tA[0:64, :])
            nc.sync.dma_start(ao[64:128, :], tB[0:64, :])
            dn = AO["dn"]
            nc.sync.dma_start(dn[4 * b + t:4 * b + t + 1, :], tA[64:65, :])
            nc.sync.dma_start(dn[4 * b + 2 + t:4 * b + 3 + t, :],
                              tB[64:65, :])
        for f in fillers:
            f()

    def finish(j, AOj):
        # normalize, build the A2A input, kick the A2A
        dn = AOj["dn"]
        dnR = evac.tile([2 * QH, QTILE], F32, name="dnR", tag="dnR")
        nc.vector.reciprocal(dnR[:], dn[:])
        dnRb = evac.tile([2 * QH, QTILE], MMD, name="dnRb", tag="dnRb")
        nc.vector.tensor_copy(dnRb[:], dnR[:])
        for b in range(B):
            for t in range(2):
                bc = psM.tile([128, QTILE], F32, name="bc", tag="mm")
                nc.tensor.matmul(
                    bc[:], sel_t[:, (2 * b + t) * 128:(2 * b + t + 1) * 128],
                    dnRb[:], start=True, stop=True)
                nc.vector.tensor_mul(AOj[b, t][:], AOj[b, t][:], bc[:])
        for d in range(NCORES):
            bd, g = d // 4, d % 4
            for t in range(2):
                nc.sync.dma_start(
                    a2a_in[j][FQ * d + 128 * t:FQ * d + 128 * (t + 1), :],
                    AOj[bd, t][:, g * KTILE:(g + 1) * KTILE])
        if single:
            nc.sync.dma_start(a2a_out[j][:], a2a_in[j][:])
        else:
            nc.gpsimd.collective_compute(
                "AllToAll", mybir.AluOpType.bypass,
                replica_groups=[list(range(NCORES))],
                ins=[a2a_in[j][:]], outs=[a2a_out[j][:]],
            )

    def make_wo_fillers(j):
        # wo for slab j, split into one gather step plus 4x4 matmul chunks
        ctx = {}

        def gather():
            ctx["aogs"] = []
            for fc in range(ND):
                aog = aogp.tile([128, KTILE], MMD, name="aog", tag=f"aog{fc}")
                q = nc.sync if fc % 2 == 0 else nc.scalar
                q.dma_start(aog[:],
                            a2a_out[j][fc * 128:(fc + 1) * 128, :])
                ctx["aogs"].append(aog)

        fillers = [gather]
        for dn in range(4):
            for sub in range(8):
                def chunk(dn=dn, sub=sub):
                    if sub == 0:
                        ctx[dn] = psM.tile([128, QTILE], F32, name="psW",
                                           tag="mm")
                    ps = ctx[dn]
                    for fc in range(2 * sub, 2 * sub + 2):
                        nc.tensor.matmul(ps[:], ctx["aogs"][fc][:],
                                         WO[fc, dn][:],
                                         start=(fc == 0), stop=(fc == ND - 1))
                    if sub == 7:
                        og = evac.tile([128, QTILE], F32, name="og", tag="og")
                        nc.vector.tensor_copy(og[:], ps[:])
                        nc.gpsimd.dma_start(
                            out_full[j * 128:(j + 1) * 128,
                                     dn * QTILE:(dn + 1) * QTILE], og[:])
                fillers.append(chunk)
        return fillers

    def wo_slab(j):
        for f in make_wo_fillers(j):
            f()

    def interleave(*lists):
        out = []
        idx = [0] * len(lists)
        while any(i < len(l) for i, l in zip(idx, lists)):
            for n, l in enumerate(lists):
                if idx[n] < len(l):
                    out.append(l[idx[n]])
                    idx[n] += 1
        return out

    proj(0, 0, xts00)
    pend = None
    for j in range(NSLAB):
        AO.clear()
        AO["dn"] = aop.tile([2 * QH, QTILE], MMD, name="dn", tag="dn")
        for b in range(B):
            AO[b, 0] = aop.tile([128, QTILE], MMD, name=f"ao{b}0",
                                tag=f"ao{b}0")
            AO[b, 1] = aop.tile([128, QTILE], MMD, name=f"ao{b}1",
                                tag=f"ao{b}1")
        if pend is not None:
            finish(*pend)
        # weave proj(1, j) into attn(0, j)'s exp-bound loop
        attn(0, j, fillers=make_proj_fillers(1, j, None if j == 0 else nxt1))
        # weave the previous slab's wo and the next slab's proj(0) into
        # attn(1, j)
        f_wo = make_wo_fillers(j - 1) if j > 0 else []
        if j + 1 < NSLAB:
            nxt0 = prefetch_x(0, j + 1)
            f_pj = make_proj_fillers(0, j + 1, nxt0)
        else:
            f_pj = []
        attn(1, j, fillers=f_pj + f_wo)
        nxt1 = prefetch_x(1, j + 1) if j + 1 < NSLAB else None
        pend = (j, dict(AO))
    finish(*pend)
    wo_slab(NSLAB - 1)

    for p in (psO, psS, psM, dram, aogp, evac, pexp, rp, aop, qpool, xpool,
              kvp, wpool, const):
        p.release()


def _build(single=False):
    nc = bacc.Bacc("TRN2", target_bir_lowering=False, debug=False,
                   num_devices=1 if single else NCORES)
    io = {
        "xT": nc.dram_tensor("xT", [B * D, S], BF16, kind="ExternalInput").ap(),
        "wq": nc.dram_tensor("wq", [D, FQ], BF16, kind="ExternalInput").ap(),
        "wkv": nc.dram_tensor("wkv", [D, FKV], BF16, kind="ExternalInput").ap(),
        "wo": nc.dram_tensor("wo", [D, D], BF16, kind="ExternalInput").ap(),
        "cos2": nc.dram_tensor("cos2", [128, S], BF16, kind="ExternalInput").ap(),
        "sin2s": nc.dram_tensor("sin2s", [128, S], BF16, kind="ExternalInput").ap(),
        "sel": nc.dram_tensor("sel", [2 * QH, 4 * KTILE], BF16,
                              kind="ExternalInput").ap(),
        "out": nc.dram_tensor("out", [NSLAB * 128, D], F32,
                              kind="ExternalOutput").ap(),
    }
    io["single"] = single
    with tile.TileContext(nc) as tc:
        _build_kernel(tc, io)
    nc.compile()
    return nc


_CACHE = {}


def _get_program():
    if "nc" not in _CACHE:
        _CACHE["nc"] = _build()
    return _CACHE["nc"]


def _host_inputs(x, wq, wk, wv, wo):
    x = np.ascontiguousarray(x, np.float32)
    inv = 1.0 / (10000.0 ** (np.arange(0, HD, 2, dtype=np.float64) / HD))
    pos = np.arange(S, dtype=np.float64)
    freqs = np.outer(pos, inv)                      # [S, 32]
    emb = np.concatenate([freqs, freqs], axis=1)    # [S, 64]
    cos = np.cos(emb).T.astype(np.float32)          # [64, S]
    sin = np.sin(emb).T.astype(np.float32)
    cos2 = np.concatenate([cos, cos], axis=0)       # [128, S]
    sin2s = np.concatenate([-sin[:32], sin[32:], -sin[:32], sin[32:]], axis=0)

    # denominator broadcast selector: for (b, t) block, AO[b,t] rows 0:64
    # <- dn row 4b+t, rows 64:128 <- dn row 4b+2+t
    sel = np.zeros((2 * QH, 4 * KTILE), np.float32)
    for b in range(2):
        for t in range(2):
            blk = (2 * b + t) * 128
            sel[4 * b + t, blk:blk + 64] = 1.0
            sel[4 * b + 2 + t, blk + 64:blk + 128] = 1.0

    import ml_dtypes
    bf16 = ml_dtypes.bfloat16
    cos2 = cos2.astype(bf16)
    sin2s = sin2s.astype(bf16)
    sel = sel.astype(bf16)
    xT = np.ascontiguousarray(
        np.concatenate([x[0].T, x[1].T], axis=0).astype(bf16))  # [2D, S]

    # wo rows ordered to match the gathered A2A feature order:
    # src core cc contributes heads (4cc+t, 4cc+t+2) for t in (0, 1)
    wrows = []
    for cc in range(NCORES):
        for t in range(2):
            for h in (4 * cc + t, 4 * cc + t + 2):
                wrows.append(wo[h * HD:(h + 1) * HD, :])
    wo_p = np.ascontiguousarray(np.concatenate(wrows, axis=0).astype(bf16))

    in_maps = []
    for c in range(NCORES):
        qcols = []
        for t in range(2):
            for h in (4 * c + t, 4 * c + t + 2):
                qcols.append(wq[:, h * HD:(h + 1) * HD])
        wq_p = np.ascontiguousarray(np.concatenate(qcols, axis=1).astype(bf16))
        wkv_p = np.ascontiguousarray(np.concatenate(
            [wk[:, c * HD:(c + 1) * HD], wv[:, c * HD:(c + 1) * HD]],
            axis=1).astype(bf16))
        in_maps.append({
            "xT": xT, "wq": wq_p, "wkv": wkv_p, "wo": wo_p,
            "cos2": cos2, "sin2s": sin2s, "sel": sel,
        })
    return in_maps


def run(x, wq, wk, wv, wo, trace=False, **trace_kwargs):
    nc = _get_program()
    in_maps = _host_inputs(x, wq, wk, wv, wo)
    res = run_bass_kernel_spmd(nc, in_maps, list(range(NCORES)),
                               trace=trace, **trace_kwargs)
    out = np.empty((B, S, D), np.float32)
    for c in range(NCORES):
        bo, g = c // 4, c % 4
        shard = res.results[c]["out"]  # [512, D]
        for j in range(NSLAB):
            out[bo, j * QTILE + g * 128:j * QTILE + (g + 1) * 128, :] = \
                shard[j * 128:(j + 1) * 128, :]
    return out, res


def kernel(x, wq, wk, wv, wo):
    out, _ = run(x, wq, wk, wv, wo)
    return out.astype(np.float32)



# revision 6
# speedup vs baseline: 1.0445x; 1.0445x over previous
"""Tensor-parallel GQA multi-head attention (RoPE + causal softmax) for 8 trn2 cores.

Sharding: every core handles BOTH batches with 4 q-heads / 1 kv-head:
core c owns q-heads {4c..4c+3} (kv-head c) of batches 0 and 1. Attention
runs in transposed (feature-major) layout with flash-style causal tiling.
Per 512-token slab, the 8 cores exchange their normalized attention outputs
with one AllToAll (bf16, 512KB) so that core c ends up with ALL 2048
attention features for its 128-position output stripe (batch c//4, stripe
c%4); it then applies the full wo to produce disjoint output rows. No
reduction collective is needed.

v3 notes (vs the 502us baseline):
- DMAs batched via 3D access patterns (x slab: 2 DMAs instead of 16; wo: 2
  instead of 64; A2A staging: 4+1 instead of 32) - each dma_start costs
  ~600ns of issuing-engine time regardless of size.
- Causal mask moved from gpsimd affine_select to one DVE bf16 multiply per
  diagonal tile (covers both heads via a [128,2,128] AP).
- RoPE half-swap done with a DVE stream_shuffle: head-dims are host-permuted
  (P64) so the rotate_half partner sits 16 lanes away within each 32-lane
  block (scores are invariant to a consistent q/k feature permutation).
- attnV evacuates directly into the AO layout (no SBUF->SBUF reshuffle
  DMAs); softmax denominators are extracted with ScalarE copies and the
  reciprocal uses the fast approx (~0.7us vs 3.3us).
- The final slab's AllToAll is hidden under reserved wo-chunks of the
  previous slab.
"""

import sys

sys.path.insert(0, "/opt/trn_rl_repo")

import numpy as np

import concourse.bass as bass
import concourse.bacc as bacc
import concourse.mybir as mybir
from concourse import tile
from concourse.bass_utils import run_bass_kernel_spmd

B, S, D = 2, 2048, 2048
N_HEADS, N_KV, HD = 32, 8, 64
NCORES = 8
QH = 4    # q-heads per core
FQ = QH * HD       # 256 q-feature cols per core
FKV = 2 * HD       # 128 (K then V) per core
SCALE = 1.0 / 8.0  # 1/sqrt(HD)

QTILE = 512
KTILE = 128
NSLAB = S // QTILE  # 4
ND = D // 128       # 16 contraction chunks

F32 = mybir.dt.float32
EXP = mybir.ActivationFunctionType.Exp
BF16 = mybir.dt.bfloat16
MMD = BF16

# rope partner sits 16 lanes away inside each 32-lane block (see P64 in
# _host_inputs)
SHUF_MASK = [(i + 16) % 32 for i in range(32)]


def _build_kernel(tc, io):
    nc = tc.nc
    xT, wq, wkv, wo = io["xT"], io["wq"], io["wkv"], io["wo"]
    cos2, sin2s, sel, tri = io["cos2"], io["sin2s"], io["sel"], io["tri"]
    out_full = io["out"]
    single = bool(io.get("single"))

    # ---------------- pools ----------------
    const = tc.alloc_tile_pool(name="const", bufs=1)
    wpool = tc.alloc_tile_pool(name="wpool", bufs=1, side="right")
    kvp = tc.alloc_tile_pool(name="kvp", bufs=1)
    xpool = tc.alloc_tile_pool(name="xpool", bufs=2)
    qpool = tc.alloc_tile_pool(name="qpool", bufs=2)
    aop = tc.alloc_tile_pool(name="aop", bufs=2, side="right")
    dsp = tc.alloc_tile_pool(name="dsp", bufs=1)
    rp = tc.alloc_tile_pool(name="rp", bufs=2)
    pexp = tc.alloc_tile_pool(name="pexp", bufs=5)
    evac = tc.alloc_tile_pool(name="evac", bufs=2)
    aogp = tc.alloc_tile_pool(name="aogp", bufs=2, side="right")
    dram = tc.alloc_tile_pool(name="dram", bufs=1, space="DRAM")

    psM = tc.alloc_tile_pool(name="psM", bufs=2, space="PSUM")
    psS = tc.alloc_tile_pool(name="psS", bufs=2, space="PSUM")
    psO = tc.alloc_tile_pool(name="psO", bufs=1, space="PSUM")

    # ------- constants + weights; DMA order tuned for fast start -------
    # first x slab (b=0, j=0) on sync; projection weights on scalar so the
    # first matmul group can start ~6us in
    def load_x(b, j):
        qs = slice(j * QTILE, (j + 1) * QTILE)
        xts = []
        for h in range(2):
            xt = xpool.tile([128, 8 * QTILE], MMD, name="xt", tag=f"xt{h}")
            r0 = b * D + h * 1024
            nc.sync.dma_start(
                xt[:].rearrange("p (k q) -> p k q", k=8),
                xT[r0:r0 + 1024, qs].rearrange("(k p) q -> p k q", p=128))
            xts.append(xt)
        return xts

    xts00 = load_x(0, 0)

    WQ = []
    for f in range(2):
        w = wpool.tile([128, ND * 128], MMD, name=f"wqt{f}")
        nc.scalar.dma_start(
            w[:].rearrange("p (k c) -> p k c", k=ND),
            wq[:, f * 128:(f + 1) * 128].rearrange("(k p) c -> p k c", p=128))
        WQ.append(w)
    WKV = wpool.tile([128, ND * 128], MMD, name="wkvt")
    nc.scalar.dma_start(
        WKV[:].rearrange("p (k c) -> p k c", k=ND),
        wkv[:, :].rearrange("(k p) c -> p k c", p=128))

    cos2_t = const.tile([128, S], MMD)
    nc.sync.dma_start(cos2_t[:], cos2[:])
    sin2s_t = const.tile([128, S], MMD)
    nc.sync.dma_start(sin2s_t[:], sin2s[:])
    sel_t = const.tile([2 * QH, 4 * KTILE], MMD)
    nc.sync.dma_start(sel_t[:], sel[:])
    tri_t = const.tile([128, 2 * KTILE], MMD)
    nc.sync.dma_start(tri_t[:], tri[:])

    ident = const.tile([128, 64], F32)
    nc.gpsimd.memset(ident[:], 0.0)
    for p in (0, 64):
        nc.gpsimd.affine_select(
            out=ident[p:p + 64, :], in_=ident[p:p + 64, :],
            compare_op=mybir.AluOpType.not_equal,
            fill=1.0, base=0, pattern=[[-1, 64]], channel_multiplier=1,
        )

    # full wo in one tile, loaded via the scalar DMA queue (idle early on)
    WO = wpool.tile([128, ND * D], MMD, name="wot")
    for h in range(2):
        nc.scalar.dma_start(
            WO[:, h * 8 * D:(h + 1) * 8 * D].rearrange(
                "p (k c) -> p k c", k=8),
            wo[h * 1024:(h + 1) * 1024, :].rearrange(
                "(k p) c -> p k c", p=128))

    # persistent K/V cache tiles
    KK = [kvp.tile([128, S], MMD, name=f"kk{b}") for b in range(B)]
    VA = {}
    for b in range(B):
        for i in range(S // KTILE):
            VA[b, i] = kvp.tile([128, HD + 1], MMD, name=f"va{b}_{i}")

    # A2A dram tiles (one pair per slab)
    a2a_in = [dram.tile([FQ * NCORES, KTILE], MMD, name=f"ain{j}")
              for j in range(NSLAB)]
    a2a_out = [dram.tile([FQ * NCORES, KTILE], MMD, name=f"aout{j}")
               for j in range(NSLAB)]

    AO = {}   # per (b, t) slab-local attention output, feature-major
    QT = {}

    def rope(dst, rows, qs, tab_qs):
        # dst[rows, qs] = dst*cos + shuffle16(dst)*sin  (feature-major RoPE;
        # the host-side P64 head-dim permutation makes the rotate_half
        # partner a +16 lane rotation within each 32-lane block)
        n = rows[1] - rows[0]
        sl = (slice(rows[0], rows[1]), qs)
        qsw = rp.tile([128, QTILE], MMD, name="qsw", tag="qsw")
        nc.vector.stream_shuffle(qsw[:n], dst[sl], mask=SHUF_MASK)
        t1 = rp.tile([128, QTILE], MMD, name="t1", tag="t1")
        nc.vector.tensor_mul(t1[:n], dst[sl], cos2_t[rows[0]:rows[1], tab_qs])
        t2 = rp.tile([128, QTILE], MMD, name="t2", tag="t2")
        nc.vector.tensor_mul(t2[:n], qsw[:n], sin2s_t[rows[0]:rows[1], tab_qs])
        nc.vector.tensor_add(dst[sl], t1[:n], t2[:n])

    def make_proj_fillers(b, j, xts):
        # projection for (b, j), split into small PE chunks so it can be
        # woven into the preceding attention's exp-bound inner loop
        qs = slice(j * QTILE, (j + 1) * QTILE)
        ctx = {}
        fillers = []
        if xts is not None:
            ctx["x"] = xts
        else:
            def loadx():
                ctx["x"] = load_x(b, j)
            fillers.append(loadx)
        for f in range(3):
            for sub in range(8):
                def mmchunk(f=f, sub=sub):
                    if sub == 0:
                        ctx[f] = psM.tile([128, QTILE], F32, name="psq",
                                          tag="mm")
                    ps = ctx[f]
                    for k in range(2 * sub, 2 * sub + 2):
                        w = WQ[f] if f < 2 else WKV
                        xt = ctx["x"][k // 8]
                        nc.tensor.matmul(
                            ps[:], w[:, k * 128:(k + 1) * 128],
                            xt[:, (k % 8) * QTILE:(k % 8 + 1) * QTILE],
                            start=(k == 0), stop=(k == ND - 1))
                fillers.append(mmchunk)

            def evacf(f=f):
                ps = ctx[f]
                if f < 2:
                    qt = qpool.tile([128, QTILE], MMD, name="qt",
                                    tag=f"qt{b}_{f}")
                    QT[b, f] = qt
                    nc.vector.tensor_copy(qt[:], ps[:])
                    rope(qt, (0, 128), slice(0, QTILE), qs)
                else:
                    nc.vector.tensor_copy(KK[b][0:64, qs], ps[0:64, :])
                    rope(KK[b], (0, 64), qs, qs)
                    # duplicate roped K into rows 64:128 (row-tiled scores)
                    nc.sync.dma_start(KK[b][64:128, qs], KK[b][0:64, qs])
                    vv = rp.tile([128, QTILE], F32, name="vv", tag="vv")
                    nc.vector.tensor_copy(vv[64:128, :], ps[64:128, :])
                    ctx["vv"] = vv
            fillers.append(evacf)
        for c in range(4):
            def vtrans(c=c):
                i = 4 * j + c
                tp = psM.tile([128, QTILE], F32, name="tp", tag="mm")
                vv = ctx["vv"]
                nc.tensor.matmul(tp[:, 0:HD],
                                 vv[64:128, c * 128:(c + 1) * 128],
                                 ident[64:128, :], is_transpose=True,
                                 start=True, stop=True)
                va = VA[b, i]
                nc.vector.tensor_copy(va[:, 0:HD], tp[:, 0:HD])
                nc.vector.memset(va[:, HD:HD + 1], 1.0)
            fillers.append(vtrans)
        return fillers

    def proj(b, j, xts=None):
        for f in make_proj_fillers(b, j, xts):
            f()

    def attn(b, j, fillers=None):
        # fillers: closures emitting small independent PE chunks (wo/proj
        # work); paced evenly through the loop and placed before each attnV
        # so the tensor engine has work while it would otherwise stall on
        # the exp
        fillers = list(fillers or [])
        nkt = 4 * j + 4
        slots = 2 * nkt
        rate = len(fillers) / slots if slots else 0.0
        acc = [0.0]

        def pop_fillers():
            acc[0] += rate
            while fillers and acc[0] >= 1.0:
                fillers.pop(0)()
                acc[0] -= 1.0
        ds = AO["ds"]
        for t in range(2):
            oA = psO.tile([HD + 1, QTILE], F32, name="oA", tag="oA")
            oB = psO.tile([HD + 1, QTILE], F32, name="oB", tag="oB")
            sabs = {}

            def scores(i):
                r = i - 4 * j
                off = max(r, 0) * KTILE
                ks = slice(i * KTILE, (i + 1) * KTILE)
                sAB = psS.tile([128, 2 * QTILE], F32, name="sAB", tag="sAB")
                nc.tensor.matmul(sAB[:, off:QTILE], KK[b][0:64, ks],
                                 QT[b, t][0:64, off:], start=True, stop=True,
                                 tile_position=(0, 0))
                nc.tensor.matmul(sAB[:, QTILE + off:], KK[b][64:128, ks],
                                 QT[b, t][64:128, off:], start=True, stop=True,
                                 tile_position=(64, 0))
                sabs[i] = sAB

            scores(0)
            for i in range(nkt):
                r = i - 4 * j
                off = max(r, 0) * KTILE
                if i + 1 < nkt:
                    scores(i + 1)
                sAB = sabs.pop(i)
                pAB = pexp.tile([128, 2 * QTILE], MMD, name="pAB", tag="pAB")
                nc.scalar.activation(pAB[:, off:], sAB[:, off:], EXP,
                                     scale=SCALE)
                if r >= 0:
                    # zero the strictly-upper triangle (causal mask) of the
                    # diagonal block for both heads in one DVE multiply
                    pv = pAB[:].rearrange("p (h q) -> p h q", h=2)
                    pv = pv[:, :, off:off + KTILE]
                    tv = tri_t[:].rearrange("p (h q) -> p h q", h=2)
                    nc.vector.tensor_mul(pv, pv, tv)
                pop_fillers()
                nc.tensor.matmul(oA[:, off:], VA[b, i][:], pAB[:, off:QTILE],
                                 start=(i == 0), stop=(i == nkt - 1))
                nc.tensor.matmul(oB[:, off:], VA[b, i][:], pAB[:, QTILE + off:],
                                 start=(i == 0), stop=(i == nkt - 1))
            # evacuate into the AO layout: head A lands on partitions 0:64
            # directly (aligned DVE copy); head B goes via a staging tile +
            # one DMA (DVE lanes cannot cross partitions); denominator rows
            # (partition 64 of the psum) via aligned ScalarE copies into ds
            ao = AO[b, t]
            nc.vector.tensor_copy(ao[0:64, :], oA[0:64, :])
            tB = evac.tile([64, QTILE], MMD, name="tB", tag="tB", bufs=3)
            nc.vector.tensor_copy(tB[:], oB[0:64, :])
            nc.sync.dma_start(ao[64:128, :], tB[:])
            rA = 4 * b + t
            rB = 4 * b + 2 + t
            nc.scalar.copy(ds[64:65, rA * QTILE:(rA + 1) * QTILE],
                           oA[64:65, :])
            nc.scalar.copy(ds[64:65, rB * QTILE:(rB + 1) * QTILE],
                           oB[64:65, :])
        for f in fillers:
            f()

    def finish(j, AOj):
        # normalize, build the A2A input, kick the A2A
        ds = AOj["ds"]
        dn = evac.tile([2 * QH, QTILE], MMD, name="dn", tag="dn")
        nc.sync.dma_start(dn[:, :], ds[64:65, 0:2 * QH * QTILE])
        dnF = evac.tile([2 * QH, QTILE], F32, name="dnF", tag="dnF")
        nc.vector.tensor_copy(dnF[:], dn[:])
        dnR = evac.tile([2 * QH, QTILE], F32, name="dnR", tag="dnR")
        nc.vector.reciprocal_approx_fast(out=dnR[:], in_=dnF[:])
        dnRb = evac.tile([2 * QH, QTILE], MMD, name="dnRb", tag="dnRb")
        nc.vector.tensor_copy(dnRb[:], dnR[:])
        # one DMA per (b, t): AO partition p goes to a2a_in row
        # 1024*b + 256*g + 128*t + p for each 128-col block g
        dst_all = a2a_in[j][:, :].rearrange(
            "(bb g t r) q -> bb t r g q", bb=2, g=4, t=2)
        for b in range(B):
            for t in range(2):
                bc = psM.tile([128, QTILE], F32, name="bc", tag="mm")
                nc.tensor.matmul(
                    bc[:], sel_t[:, (2 * b + t) * 128:(2 * b + t + 1) * 128],
                    dnRb[:], start=True, stop=True)
                nc.vector.tensor_mul(AOj[b, t][:], AOj[b, t][:], bc[:])
                nc.sync.dma_start(
                    dst_all[b:b + 1, t:t + 1],
                    AOj[b, t][:, :].rearrange("p (g q) -> p g q", g=4))
        if single:
            nc.sync.dma_start(a2a_out[j][:], a2a_in[j][:])
        else:
            nc.gpsimd.collective_compute(
                "AllToAll", mybir.AluOpType.bypass,
                replica_groups=[list(range(NCORES))],
                ins=[a2a_in[j][:]], outs=[a2a_out[j][:]],
            )

    def make_wo_fillers(j):
        # wo for slab j: one batched gather + 4x8 matmul chunks; the output
        # stripe is assembled in og and stored with a single gpsimd DMA
        ctx = {}

        def gather():
            aog = aogp.tile([128, ND * KTILE], MMD, name="aog", tag="aog")
            nc.sync.dma_start(
                aog[:].rearrange("p (k q) -> p k q", k=ND),
                a2a_out[j][:, :].rearrange("(k p) q -> p k q", p=128))
            ctx["aog"] = aog
            ctx["og"] = evac.tile([128, D], F32, name="og", tag="og", bufs=1)

        fillers = [gather]
        for dn_ in range(4):
            for sub in range(8):
                def chunk(dn_=dn_, sub=sub):
                    if sub == 0:
                        ctx[dn_] = psM.tile([128, QTILE], F32, name="psW",
                                            tag="mm")
                    ps = ctx[dn_]
                    aog = ctx["aog"]
                    for fc in range(2 * sub, 2 * sub + 2):
                        nc.tensor.matmul(
                            ps[:], aog[:, fc * KTILE:(fc + 1) * KTILE],
                            WO[:, fc * D + dn_ * QTILE:
                               fc * D + (dn_ + 1) * QTILE],
                            start=(fc == 0), stop=(fc == ND - 1))
                    if sub == 7:
                        og = ctx["og"]
                        nc.vector.tensor_copy(
                            og[:, dn_ * QTILE:(dn_ + 1) * QTILE], ps[:])
                        if dn_ == 3:
                            nc.gpsimd.dma_start(
                                out_full[j * 128:(j + 1) * 128, :], og[:])
                fillers.append(chunk)
        return fillers

    def wo_slab(j):
        for f in make_wo_fillers(j):
            f()

    proj(0, 0, xts00)
    pend = None
    reserved = []
    for j in range(NSLAB):
        AO.clear()
        AO["ds"] = dsp.tile([65, 2 * QH * QTILE], MMD, name="ds", tag="ds")
        for b in range(B):
            AO[b, 0] = aop.tile([128, QTILE], MMD, name=f"ao{b}0",
                                tag=f"ao{b}0")
            AO[b, 1] = aop.tile([128, QTILE], MMD, name=f"ao{b}1",
                                tag=f"ao{b}1")
        if pend is not None:
            finish(*pend)
        # weave proj(1, j) into attn(0, j)'s exp-bound loop
        attn(0, j, fillers=make_proj_fillers(1, j, None if j == 0 else nxt1))
        # weave the previous slab's wo and the next slab's proj(0) into
        # attn(1, j); at the last slab hold back some wo chunks to hide the
        # final A2A
        f_wo = make_wo_fillers(j - 1) if j > 0 else []
        if j == NSLAB - 1:
            f_wo, reserved = f_wo[:21], f_wo[21:]
        if j + 1 < NSLAB:
            nxt0 = load_x(0, j + 1)
            f_pj = make_proj_fillers(0, j + 1, nxt0)
        else:
            f_pj = []
        attn(1, j, fillers=f_pj + f_wo)
        nxt1 = load_x(1, j + 1) if j + 1 < NSLAB else None
        pend = (j, dict(AO))
    finish(*pend)
    for f in reserved:
        f()
    wo_slab(NSLAB - 1)

    for p in (psO, psS, psM, dram, aogp, evac, pexp, rp, dsp, aop, qpool,
              xpool, kvp, wpool, const):
        p.release()


def _build(single=False):
    nc = bacc.Bacc("TRN2", target_bir_lowering=False, debug=False,
                   num_devices=1 if single else NCORES)
    io = {
        "xT": nc.dram_tensor("xT", [B * D, S], BF16, kind="ExternalInput").ap(),
        "wq": nc.dram_tensor("wq", [D, FQ], BF16, kind="ExternalInput").ap(),
        "wkv": nc.dram_tensor("wkv", [D, FKV], BF16, kind="ExternalInput").ap(),
        "wo": nc.dram_tensor("wo", [D, D], BF16, kind="ExternalInput").ap(),
        "cos2": nc.dram_tensor("cos2", [128, S], BF16, kind="ExternalInput").ap(),
        "sin2s": nc.dram_tensor("sin2s", [128, S], BF16, kind="ExternalInput").ap(),
        "sel": nc.dram_tensor("sel", [2 * QH, 4 * KTILE], BF16,
                              kind="ExternalInput").ap(),
        "tri": nc.dram_tensor("tri", [128, 2 * KTILE], BF16,
                              kind="ExternalInput").ap(),
        "out": nc.dram_tensor("out", [NSLAB * 128, D], F32,
                              kind="ExternalOutput").ap(),
    }
    io["single"] = single
    with tile.TileContext(nc) as tc:
        _build_kernel(tc, io)
    nc.compile()
    return nc


_CACHE = {}


def _get_program():
    if "nc" not in _CACHE:
        _CACHE["nc"] = _build()
    return _CACHE["nc"]


# head-dim permutation: pairs (d, d+32) end up 16 lanes apart within each
# 32-lane block, so rotate_half becomes a +16 lane rotation (stream_shuffle)
P64 = np.concatenate([np.arange(0, 16), np.arange(32, 48),
                      np.arange(16, 32), np.arange(48, 64)])


def _host_inputs(x, wq, wk, wv, wo):
    x = np.ascontiguousarray(x, np.float32)
    inv = 1.0 / (10000.0 ** (np.arange(0, HD, 2, dtype=np.float64) / HD))
    pos = np.arange(S, dtype=np.float64)
    freqs = np.outer(pos, inv)                      # [S, 32]
    emb = np.concatenate([freqs, freqs], axis=1)    # [S, 64]
    cos = np.cos(emb).T.astype(np.float32)          # [64, S]
    sin = np.sin(emb).T.astype(np.float32)
    sin2 = np.concatenate([-sin[:32], sin[32:]], axis=0)  # [64, S]
    cosP, sin2P = cos[P64], sin2[P64]
    cos2 = np.concatenate([cosP, cosP], axis=0)       # [128, S]
    sin2s = np.concatenate([sin2P, sin2P], axis=0)

    # denominator broadcast selector: for (b, t) block, AO[b,t] rows 0:64
    # <- dn row 4b+t, rows 64:128 <- dn row 4b+2+t
    sel = np.zeros((2 * QH, 4 * KTILE), np.float32)
    for b in range(2):
        for t in range(2):
            blk = (2 * b + t) * 128
            sel[4 * b + t, blk:blk + 64] = 1.0
            sel[4 * b + 2 + t, blk + 64:blk + 128] = 1.0

    # causal keep-mask for a 128x128 diagonal block (key=partition p kept
    # when local query c >= p), duplicated for the two heads
    tri1 = (np.arange(KTILE)[None, :] >= np.arange(KTILE)[:, None])
    tri = np.concatenate([tri1, tri1], axis=1).astype(np.float32)

    import ml_dtypes
    bf16 = ml_dtypes.bfloat16
    cos2 = cos2.astype(bf16)
    sin2s = sin2s.astype(bf16)
    sel = sel.astype(bf16)
    tri = tri.astype(bf16)
    xT = np.ascontiguousarray(
        np.concatenate([x[0].T, x[1].T], axis=0).astype(bf16))  # [2D, S]

    # wo rows ordered to match the gathered A2A feature order:
    # src core cc contributes heads (4cc+t, 4cc+t+2) for t in (0, 1)
    wrows = []
    for cc in range(NCORES):
        for t in range(2):
            for h in (4 * cc + t, 4 * cc + t + 2):
                wrows.append(wo[h * HD:(h + 1) * HD, :])
    wo_p = np.ascontiguousarray(np.concatenate(wrows, axis=0).astype(bf16))

    in_maps = []
    for c in range(NCORES):
        qcols = []
        for t in range(2):
            for h in (4 * c + t, 4 * c + t + 2):
                qcols.append(wq[:, h * HD:(h + 1) * HD][:, P64])
        wq_p = np.ascontiguousarray(np.concatenate(qcols, axis=1).astype(bf16))
        wkv_p = np.ascontiguousarray(np.concatenate(
            [wk[:, c * HD:(c + 1) * HD][:, P64],
             wv[:, c * HD:(c + 1) * HD]],
            axis=1).astype(bf16))
        in_maps.append({
            "xT": xT, "wq": wq_p, "wkv": wkv_p, "wo": wo_p,
            "cos2": cos2, "sin2s": sin2s, "sel": sel, "tri": tri,
        })
    return in_maps


def run(x, wq, wk, wv, wo, trace=False, **trace_kwargs):
    nc = _get_program()
    in_maps = _host_inputs(x, wq, wk, wv, wo)
    res = run_bass_kernel_spmd(nc, in_maps, list(range(NCORES)),
                               trace=trace, **trace_kwargs)
    out = np.empty((B, S, D), np.float32)
    for c in range(NCORES):
        bo, g = c // 4, c % 4
        shard = res.results[c]["out"]  # [512, D]
        for j in range(NSLAB):
            out[bo, j * QTILE + g * 128:j * QTILE + (g + 1) * 128, :] = \
                shard[j * 128:(j + 1) * 128, :]
    return out, res


def kernel(x, wq, wk, wv, wo):
    out, _ = run(x, wq, wk, wv, wo)
    return out.astype(np.float32)
